# revision 28
# baseline (speedup 1.0000x reference)
"""LEM cell (ODE2) Bass kernel for Trainium2, 8-core data-parallel.

Math (per batch row b):
  ti = x @ W_ih.T + b_ih                  # [B, 4H]
  th = y @ W_hh.T + b_hh                  # [B, 3H]
  tdt = dt @ W_dt.T + b_dt                # [B, 2]
  ms_dt_bar = sig(tdt[:,0]) * sig(ti[:, :H]   + th[:, :H])
  ms_dt     = sig(tdt[:,1]) * sig(ti[:, H:2H] + th[:, H:2H])
  z_new = (1-ms_dt) * z + ms_dt * tanh(ti[:, 3H:] + th[:, 2H:3H])
  y_new = (1-ms_dt_bar) * y + ms_dt_bar * tanh(z_new @ W_z.T + b_z + ti[:, 2H:3H])
  returns (y_new, z_new)

Strategy: shard batch across 8 cores (2048 rows each). On-chip everything is
feature-major ([feature_tile=128 partitions, batch columns free]) so no
on-chip transposes are needed: the host pre-transposes x/y/z and pre-packs
the weights into per-output-tile stationary blocks. The i+h sums and the
i_z + z_new@W_z.T sum are obtained for free by accumulating both GEMMs into
the same PSUM bank. Matmuls run as float32r (fp32 bits, full PE rate;
HW rounds internally, ~1.5e-4 rel per K=128 tile).
"""

import sys

_REPO = "/opt/trn_rl_repo"
if _REPO not in sys.path:
    sys.path.insert(0, _REPO)

from contextlib import ExitStack

import numpy as np

import concourse.bacc as bacc
import concourse.bass as bass
import concourse.tile as tile
from concourse import mybir
from concourse.bass_utils import run_bass_kernel_spmd

P = 128
F32 = mybir.dt.float32
F32R = mybir.dt.float32r
AF = mybir.ActivationFunctionType

N_CORES = 8
NINP = 1024
NHID = 1024
BATCH = 16384

LAST_RESULTS = None  # BassKernelResults of the most recent kernel() call


def build_nc(
    K,            # input feature dim (x)
    H,            # hidden dim (y/z)
    B_shard,      # batch rows per core
    panel,        # batch columns kept resident per pass
    chunk,        # matmul moving-dim size (<=512 fp32)
    wdt00, wdt10,  # W_dt scalars (baked immediates; b_dt rides in biasP)
    mm_dt=F32R,
    w_bufs=4,
    ps_bufs=8,
    xy_bufs=None,
):
    NJT = H // P          # output feature tiles (per H-sized group)
    NKT = K // P          # contraction tiles over x features
    NHT = H // P          # contraction tiles over y/z features
    npan = B_shard // panel
    nch = panel // chunk
    if xy_bufs is None:
        xy_bufs = NKT * nch + 6   # one panel's tiles + cross-panel prefetch

    def f32v(ap):
        """fp32 view of an mm-typed AP for DVE/ACT consumers."""
        return ap.bitcast(F32) if mm_dt != F32 else ap

    nc = bacc.Bacc(trn_type="TRN2", target_bir_lowering=False)

    xT = nc.declare_dram_parameter("xT", [K, B_shard], mm_dt, isOutput=False)
    yT = nc.declare_dram_parameter("yT", [H, B_shard], mm_dt, isOutput=False)
    zT = nc.declare_dram_parameter("zT", [H, B_shard], F32, isOutput=False)
    dtr = nc.declare_dram_parameter("dtr", [1, B_shard], F32, isOutput=False)
    # packed stationary blocks: [jt, kin, (kt_a*P+j | kt_b*P+j)]
    Wd2 = nc.declare_dram_parameter("Wd2", [NJT, P, K + H], mm_dt, isOutput=False)
    Wy = nc.declare_dram_parameter("Wy", [NJT, P, K + H], mm_dt, isOutput=False)
    Wd1 = nc.declare_dram_parameter("Wd1", [NJT, P, K + H], mm_dt, isOutput=False)
    Wg3 = nc.declare_dram_parameter("Wg3", [NJT, P, K + H], mm_dt, isOutput=False)
    # last two columns: row 0 holds b_dt[0], b_dt[1]
    biasP = nc.declare_dram_parameter("biasP", [P, 4 * NJT + 2], F32, isOutput=False)

    y_newT = nc.declare_dram_parameter("y_newT", [H, B_shard], F32, isOutput=True)
    z_newT = nc.declare_dram_parameter("z_newT", [H, B_shard], F32, isOutput=True)

    with tile.TileContext(nc) as tc, ExitStack() as ctx:
        cpool = ctx.enter_context(tc.tile_pool(name="cpool", bufs=1))
        xpool = ctx.enter_context(tc.tile_pool(name="xpool", bufs=xy_bufs))
        ypool = ctx.enter_context(tc.tile_pool(name="ypool", bufs=xy_bufs))
        zpool = ctx.enter_context(tc.tile_pool(name="zpool", bufs=4))
        znpool = ctx.enter_context(tc.tile_pool(name="znpool", bufs=NHT))
        wpool = ctx.enter_context(tc.tile_pool(name="wpool", bufs=w_bufs))
        apool = ctx.enter_context(tc.tile_pool(name="apool", bufs=3))
        dpool = ctx.enter_context(tc.tile_pool(name="dpool", bufs=4))
        opool = ctx.enter_context(tc.tile_pool(name="opool", bufs=2))
        bcpool = ctx.enter_context(tc.tile_pool(name="bcpool", bufs=1))
        rpool = ctx.enter_context(tc.tile_pool(name="rpool", bufs=2))
        pspool = ctx.enter_context(tc.tile_pool(name="pspool", bufs=ps_bufs, space="PSUM"))

        bias_sb = cpool.tile([P, 4 * NJT + 2], F32, name="bias_sb")
        nc.sync.dma_start(bias_sb[:], biasP[:, :])

        def bias_ap(g, jt):
            i = g * NJT + jt
            return bias_sb[:, i : i + 1]

        for p in range(npan):
            b0 = p * panel

            def col(c, n=1):
                return slice(b0 + c * chunk, b0 + (c + n) * chunk)

            # chunked input tiles; cold-start-friendly DMA order:
            # sync: dt, first weights; scalar: x(c0), y(c0), then c1
            dt_sb = rpool.tile([1, panel], F32, name="dt_sb", tag="dtr", bufs=1)
            nc.sync.dma_start(dt_sb[:], dtr[0:1, b0 : b0 + panel])

            # per-batch dt gates first: tiny ACT ops must precede the input
            # DMA flood in the ACT FIFO, else bc gates arrive ~40us late
            sg1 = rpool.tile([1, panel], F32, name="sg1", tag="sg")
            nc.scalar.activation(
                sg1[:], dt_sb[:], AF.Sigmoid,
                bias=bias_sb[0:1, 4 * NJT : 4 * NJT + 1], scale=wdt00,
            )
            sg2 = rpool.tile([1, panel], F32, name="sg2", tag="sg")
            nc.scalar.activation(
                sg2[:], dt_sb[:], AF.Sigmoid,
                bias=bias_sb[0:1, 4 * NJT + 1 : 4 * NJT + 2], scale=wdt10,
            )
            bc1 = bcpool.tile([P, panel], F32, name="bc1", tag="bc1")
            nc.gpsimd.partition_broadcast(bc1[:], sg1[0:1, :])
            bc2 = bcpool.tile([P, panel], F32, name="bc2", tag="bc2")
            nc.gpsimd.partition_broadcast(bc2[:], sg2[0:1, :])

            x_t = [[None] * nch for _ in range(NKT)]
            y_t = [[None] * nch for _ in range(NHT)]

            def load_x(kt, c):
                xt_ = xpool.tile([P, chunk], mm_dt, name="xt", tag="xt")
                nc.scalar.dma_start(xt_[:], xT[kt * P : (kt + 1) * P, col(c)])
                x_t[kt][c] = xt_

            def load_y(kt, c):
                yt_ = ypool.tile([P, chunk], mm_dt, name="yt", tag="yt")
                nc.sync.dma_start(yt_[:], yT[kt * P : (kt + 1) * P, col(c)])
                y_t[kt][c] = yt_

            def load_w(Wsrc, jt, name):
                w_sb = wpool.tile([P, K + H], mm_dt, name=name, tag="w")
                nc.sync.dma_start(w_sb[:, 0:K], Wsrc[jt][:, 0:K])
                nc.scalar.dma_start(w_sb[:, K : K + H], Wsrc[jt][:, K : K + H])
                return w_sb

            def load_w_half(Wsrc, jt, w_sb, half):
                eng = nc.sync if half == 0 else nc.scalar
                lo = 0 if half == 0 else K
                hi = K if half == 0 else K + H
                eng.dma_start(w_sb[:, lo:hi], Wsrc[jt][:, lo:hi])

            # staged cold-start: the ih halves (sync) land before y(c0),
            # the hh halves (scalar) after x(c0); two jt's worth prestaged
            n_pre = min(2, NJT)
            pre_w = []
            for jt in range(n_pre):
                wd2_sb = wpool.tile([P, K + H], mm_dt, name="wd2_sb", tag="w")
                wy_sb = wpool.tile([P, K + H], mm_dt, name="wy_sb", tag="w")
                pre_w.append((wd2_sb, wy_sb))
            for idx in range(max(n_pre, nch)):
                if idx < n_pre:
                    load_w_half(Wd2, idx, pre_w[idx][0], 0)
                    load_w_half(Wy, idx, pre_w[idx][1], 0)
                if idx < nch:
                    for kt in range(NKT):
                        load_x(kt, idx)
                if idx < n_pre:
                    load_w_half(Wd2, idx, pre_w[idx][0], 1)
                    load_w_half(Wy, idx, pre_w[idx][1], 1)
                if idx < nch:
                    for kt in range(NHT):
                        load_y(kt, idx)

            def accum_group(ps, w_sb, rhs_a, rhs_b):
                """16-matmul accumulation: sum_k Wa[k].T@a[k] + Wb[k].T@b[k]."""
                n_a = len(rhs_a)
                for kt in range(n_a):
                    nc.tensor.matmul(
                        ps[:],
                        lhsT=w_sb[:, kt * P : (kt + 1) * P],
                        rhs=rhs_a[kt][:],
                        start=(kt == 0),
                        stop=False,
                    )
                n_b = len(rhs_b)
                for kt in range(n_b):
                    nc.tensor.matmul(
                        ps[:],
                        lhsT=w_sb[:, K + kt * P : K + (kt + 1) * P],
                        rhs=rhs_b[kt][:],
                        start=False,
                        stop=(kt == n_b - 1),
                    )

            # ---- phase B: d2 + y gates -> z_new ----
            # first two jt's interleave their c-iterations (c0 first) so the
            # cold DMA window only has to deliver chunk-0 inputs up front
            zn_t = [None] * NJT
            w_b = [None] * NJT
            b_order = []
            for c in range(nch):
                for jt in range(n_pre):
                    b_order.append((jt, c))
            for jt in range(n_pre, NJT):
                for c in range(nch):
                    b_order.append((jt, c))
            for jt, c in b_order:
                if jt < n_pre:
                    wd2_sb, wy_sb = pre_w[jt]
                else:
                    if w_b[jt] is None:
                        w_b[jt] = (load_w(Wd2, jt, "wd2_sb"), load_w(Wy, jt, "wy_sb"))
                    wd2_sb, wy_sb = w_b[jt]
                if zn_t[jt] is None:
                    zn_t[jt] = znpool.tile([P, panel], mm_dt, name="znr", tag="zn")
                znr = zn_t[jt]
                if True:
                    cs = slice(c * chunk, (c + 1) * chunk)
                    z_sb = zpool.tile([P, chunk], F32, name="z_sb", tag="z")
                    nc.gpsimd.dma_start(z_sb[:], zT[jt * P : (jt + 1) * P, col(c)])

                    ps1 = pspool.tile([P, chunk], F32, name="ps1", tag="ps")
                    accum_group(ps1, wd2_sb, [x_t[k][c] for k in range(NKT)],
                                [y_t[k][c] for k in range(NHT)])
                    s2 = apool.tile([P, chunk], F32, name="s2", tag="act")
                    nc.scalar.activation(s2[:], ps1[:], AF.Sigmoid, bias=bias_ap(0, jt), scale=1.0)

                    ps2 = pspool.tile([P, chunk], F32, name="ps2", tag="ps")
                    accum_group(ps2, wy_sb, [x_t[k][c] for k in range(NKT)],
                                [y_t[k][c] for k in range(NHT)])
                    tz = apool.tile([P, chunk], F32, name="tz", tag="act")
                    nc.scalar.activation(tz[:], ps2[:], AF.Tanh, bias=bias_ap(1, jt), scale=1.0)

                    ms2 = dpool.tile([P, chunk], F32, name="ms2", tag="dve")
                    nc.vector.tensor_mul(ms2[:], s2[:], bc2[:, cs])
                    dlt = dpool.tile([P, chunk], F32, name="dlt", tag="dve")
                    nc.vector.tensor_sub(dlt[:], tz[:], z_sb[:])
                    prd = dpool.tile([P, chunk], F32, name="prd", tag="dve")
                    nc.vector.tensor_mul(prd[:], ms2[:], dlt[:])
                    znc = opool.tile([P, chunk], F32, name="znc", tag="znc")
                    nc.vector.tensor_add(znc[:], prd[:], z_sb[:])
                    nc.sync.dma_start(
                        z_newT[jt * P : (jt + 1) * P, col(c)], znc[:]
                    )
                    # rounding cast into the resident fp32r tile for GEMM3
                    nc.gpsimd.dma_start(znr[:, cs], znc[:])

            # ---- phase C: d1 gate + (i_z + z_new @ W_z.T) -> y_new ----
            for jt in range(NJT):
                wd1_sb = load_w(Wd1, jt, "wd1_sb")
                wg3_sb = load_w(Wg3, jt, "wg3_sb")
                for c in range(nch):
                    cs = slice(c * chunk, (c + 1) * chunk)
                    ps3 = pspool.tile([P, chunk], F32, name="ps3", tag="ps")
                    accum_group(ps3, wd1_sb, [x_t[k][c] for k in range(NKT)],
                                [y_t[k][c] for k in range(NHT)])
                    s1 = apool.tile([P, chunk], F32, name="s1", tag="act")
                    nc.scalar.activation(s1[:], ps3[:], AF.Sigmoid, bias=bias_ap(2, jt), scale=1.0)

                    ps4 = pspool.tile([P, chunk], F32, name="ps4", tag="ps")
                    accum_group(ps4, wg3_sb, [x_t[k][c] for k in range(NKT)],
                                [zn_t[h][:, cs] for h in range(NHT)])
                    u = apool.tile([P, chunk], F32, name="u", tag="act")
                    nc.scalar.activation(u[:], ps4[:], AF.Tanh, bias=bias_ap(3, jt), scale=1.0)

                    # yn = (y - ms1*y) + ms1*u; the first two ops only
                    # need s1, so just two DVE ops trail the final tanh
                    ms1 = dpool.tile([P, chunk], F32, name="ms1", tag="dve")
                    nc.vector.tensor_mul(ms1[:], s1[:], bc1[:, cs])
                    my = dpool.tile([P, chunk], F32, name="my", tag="dve")
                    nc.vector.tensor_mul(my[:], ms1[:], f32v(y_t[jt][c][:]))
                    wyp = dpool.tile([P, chunk], F32, name="wyp", tag="dve")
                    nc.vector.tensor_sub(wyp[:], f32v(y_t[jt][c][:]), my[:])
                    mu = dpool.tile([P, chunk], F32, name="mu", tag="dve")
                    nc.vector.tensor_mul(mu[:], ms1[:], u[:])
                    yn = opool.tile([P, chunk], F32, name="yn", tag="yn")
                    nc.vector.tensor_add(yn[:], wyp[:], mu[:])
                    nc.scalar.dma_start(
                        y_newT[jt * P : (jt + 1) * P, col(c)], yn[:]
                    )

    nc.compile()
    return nc


def _pack_pair(Wa, Wb):
    """[jt, kin, kt*P+j] stationary-block packing of two row-major [out, in]
    weight matrices (lhsT blocks: lhsT[kin, j] = W[jt*P+j, kt*P+kin])."""
    def pack(W):
        O, I = W.shape
        njt, nkt = O // P, I // P
        return (
            W.reshape(njt, P, nkt, P).transpose(0, 3, 2, 1).reshape(njt, P, I)
        )
    A = pack(Wa)
    B = pack(Wb)
    return np.ascontiguousarray(np.concatenate([A, B], axis=2), dtype=np.float32)


def pack_host_inputs(x, y, z, dt, W_ih, b_ih, W_hh, b_hh, W_z, b_z, b_dt, n_cores):
    """Shard batch across cores; pre-transpose activations; pack weights."""
    B, K = x.shape
    H = y.shape[1]
    NJT = H // P
    Bs = B // n_cores

    xT = np.ascontiguousarray(x.T)
    yT = np.ascontiguousarray(y.T)
    zT = np.ascontiguousarray(z.T)
    dtrow = np.ascontiguousarray(dt.reshape(1, B))

    Wd2 = _pack_pair(W_ih[H : 2 * H], W_hh[H : 2 * H])
    Wy = _pack_pair(W_ih[3 * H : 4 * H], W_hh[2 * H : 3 * H])
    Wd1 = _pack_pair(W_ih[0:H], W_hh[0:H])
    Wg3 = _pack_pair(W_ih[2 * H : 3 * H], W_z)

    def bias_cols(bvec):
        return bvec.reshape(NJT, P).T  # [P, NJT]

    bdt_cols = np.zeros((P, 2), np.float32)
    bdt_cols[0, 0] = b_dt[0]
    bdt_cols[0, 1] = b_dt[1]
    biasP = np.ascontiguousarray(
        np.concatenate(
            [
                bias_cols(b_ih[H : 2 * H] + b_hh[H : 2 * H]),
                bias_cols(b_ih[3 * H : 4 * H] + b_hh[2 * H : 3 * H]),
                bias_cols(b_ih[0:H] + b_hh[0:H]),
                bias_cols(b_ih[2 * H : 3 * H] + b_z),
                bdt_cols,
            ],
            axis=1,
        ),
        dtype=np.float32,
    )

    in_maps = []
    for c in range(n_cores):
        cs = slice(c * Bs, (c + 1) * Bs)
        in_maps.append(
            {
                "xT": np.ascontiguousarray(xT[:, cs]),
                "yT": np.ascontiguousarray(yT[:, cs]),
                "zT": np.ascontiguousarray(zT[:, cs]),
                "dtr": np.ascontiguousarray(dtrow[:, cs]),
                "Wd2": Wd2,
                "Wy": Wy,
                "Wd1": Wd1,
                "Wg3": Wg3,
                "biasP": biasP,
            }
        )
    return in_maps


def kernel(x, y, z, dt, W_ih, b_ih, W_hh, b_hh, W_z, b_z, W_dt, b_dt):
    x = np.asarray(x, np.float32)
    y = np.asarray(y, np.float32)
    z = np.asarray(z, np.float32)
    dt = np.asarray(dt, np.float32)
    W_ih = np.asarray(W_ih, np.float32)
    b_ih = np.asarray(b_ih, np.float32)
    W_hh = np.asarray(W_hh, np.float32)
    b_hh = np.asarray(b_hh, np.float32)
    W_z = np.asarray(W_z, np.float32)
    b_z = np.asarray(b_z, np.float32)
    W_dt = np.asarray(W_dt, np.float32)
    b_dt = np.asarray(b_dt, np.float32)

    B, K = x.shape
    H = y.shape[1]
    Bs = B // N_CORES

    in_maps = pack_host_inputs(
        x, y, z, dt, W_ih, b_ih, W_hh, b_hh, W_z, b_z, b_dt, N_CORES
    )
    nc = build_nc(
        K,
        H,
        Bs,
        panel=1024,
        chunk=512,
        wdt00=float(W_dt[0, 0]),
        wdt10=float(W_dt[1, 0]),
    )
    import os

    trace = os.environ.get("LEM_TRACE", "0") == "1"
    tmpdir = os.environ.get("LEM_TMPDIR") or None
    res = run_bass_kernel_spmd(
        nc, in_maps, list(range(N_CORES)), trace=trace, tmpdir=tmpdir
    )
    global LAST_RESULTS
    LAST_RESULTS = res
    y_newT = np.concatenate([r["y_newT"] for r in res.results], axis=1)
    z_newT = np.concatenate([r["z_newT"] for r in res.results], axis=1)
    return (
        np.ascontiguousarray(y_newT.T, dtype=np.float32),
        np.ascontiguousarray(z_newT.T, dtype=np.float32),
    )


# revision 30
# speedup vs baseline: 1.0545x; 1.0545x over previous
"""LEM cell (ODE2) Bass kernel for Trainium2, 8-core data-parallel.

Math (per batch row b):
  ti = x @ W_ih.T + b_ih                  # [B, 4H]
  th = y @ W_hh.T + b_hh                  # [B, 3H]
  tdt = dt @ W_dt.T + b_dt                # [B, 2]
  ms_dt_bar = sig(tdt[:,0]) * sig(ti[:, :H]   + th[:, :H])
  ms_dt     = sig(tdt[:,1]) * sig(ti[:, H:2H] + th[:, H:2H])
  z_new = (1-ms_dt) * z + ms_dt * tanh(ti[:, 3H:] + th[:, 2H:3H])
  y_new = (1-ms_dt_bar) * y + ms_dt_bar * tanh(z_new @ W_z.T + b_z + ti[:, 2H:3H])
  returns (y_new, z_new)

Strategy: shard batch across 8 cores (2048 rows each). On-chip everything is
feature-major ([feature_tile=128 partitions, batch columns free]) so no
on-chip transposes are needed: the host pre-transposes x/y/z and pre-packs
the weights into per-output-tile stationary blocks. The i+h sums and the
i_z + z_new@W_z.T sum are obtained for free by accumulating both GEMMs into
the same PSUM bank. Matmuls run as float32r (fp32 bits, full PE rate;
HW rounds internally, ~1.5e-4 rel per K=128 tile).
"""

import sys

_REPO = "/opt/trn_rl_repo"
if _REPO not in sys.path:
    sys.path.insert(0, _REPO)

from contextlib import ExitStack

import numpy as np

import concourse.bacc as bacc
import concourse.bass as bass
import concourse.tile as tile
from concourse import mybir
from concourse.bass_utils import run_bass_kernel_spmd

P = 128
F32 = mybir.dt.float32
F32R = mybir.dt.float32r
AF = mybir.ActivationFunctionType

N_CORES = 8
NINP = 1024
NHID = 1024
BATCH = 16384

LAST_RESULTS = None  # BassKernelResults of the most recent kernel() call


def build_nc(
    K,            # input feature dim (x)
    H,            # hidden dim (y/z)
    B_shard,      # batch rows per core
    panel,        # batch columns kept resident per pass
    chunk,        # matmul moving-dim size (<=512 fp32)
    wdt00, wdt10,  # W_dt scalars (baked immediates; b_dt rides in biasP)
    mm_dt=F32R,
    w_bufs=5,
    ps_bufs=8,
    xy_bufs=None,
):
    NJT = H // P          # output feature tiles (per H-sized group)
    NKT = K // P          # contraction tiles over x features
    NHT = H // P          # contraction tiles over y/z features
    npan = B_shard // panel
    nch = panel // chunk
    if xy_bufs is None:
        xy_bufs = NKT * nch + 6   # one panel's tiles + cross-panel prefetch

    def f32v(ap):
        """fp32 view of an mm-typed AP for DVE/ACT consumers."""
        return ap.bitcast(F32) if mm_dt != F32 else ap

    nc = bacc.Bacc(trn_type="TRN2", target_bir_lowering=False)

    xT = nc.declare_dram_parameter("xT", [K, B_shard], mm_dt, isOutput=False)
    yT = nc.declare_dram_parameter("yT", [H, B_shard], mm_dt, isOutput=False)
    zT = nc.declare_dram_parameter("zT", [H, B_shard], F32, isOutput=False)
    dtr = nc.declare_dram_parameter("dtr", [1, B_shard], F32, isOutput=False)
    # packed stationary blocks: [jt, kin, (kt_a*P+j | kt_b*P+j)]
    Wd2 = nc.declare_dram_parameter("Wd2", [NJT, P, K + H], mm_dt, isOutput=False)
    Wy = nc.declare_dram_parameter("Wy", [NJT, P, K + H], mm_dt, isOutput=False)
    Wd1 = nc.declare_dram_parameter("Wd1", [NJT, P, K + H], mm_dt, isOutput=False)
    Wg3 = nc.declare_dram_parameter("Wg3", [NJT, P, K + H], mm_dt, isOutput=False)
    # last two columns: row 0 holds b_dt[0], b_dt[1]
    biasP = nc.declare_dram_parameter("biasP", [P, 4 * NJT + 2], F32, isOutput=False)

    y_newT = nc.declare_dram_parameter("y_newT", [H, B_shard], F32, isOutput=True)
    z_newT = nc.declare_dram_parameter("z_newT", [H, B_shard], F32, isOutput=True)

    with tile.TileContext(nc) as tc, ExitStack() as ctx:
        cpool = ctx.enter_context(tc.tile_pool(name="cpool", bufs=1))
        xpool = ctx.enter_context(tc.tile_pool(name="xpool", bufs=xy_bufs))
        ypool = ctx.enter_context(tc.tile_pool(name="ypool", bufs=xy_bufs))
        zpool = ctx.enter_context(tc.tile_pool(name="zpool", bufs=2))
        znpool = ctx.enter_context(tc.tile_pool(name="znpool", bufs=NHT))
        wpool = ctx.enter_context(tc.tile_pool(name="wpool", bufs=w_bufs))
        apool = ctx.enter_context(tc.tile_pool(name="apool", bufs=3))
        dpool = ctx.enter_context(tc.tile_pool(name="dpool", bufs=4))
        opool = ctx.enter_context(tc.tile_pool(name="opool", bufs=2))
        bcpool = ctx.enter_context(tc.tile_pool(name="bcpool", bufs=1))
        rpool = ctx.enter_context(tc.tile_pool(name="rpool", bufs=2))
        pspool = ctx.enter_context(tc.tile_pool(name="pspool", bufs=ps_bufs, space="PSUM"))

        bias_sb = cpool.tile([P, 4 * NJT + 2], F32, name="bias_sb")
        nc.sync.dma_start(bias_sb[:], biasP[:, :])

        def bias_ap(g, jt):
            i = g * NJT + jt
            return bias_sb[:, i : i + 1]

        for p in range(npan):
            b0 = p * panel

            def col(c, n=1):
                return slice(b0 + c * chunk, b0 + (c + n) * chunk)

            # chunked input tiles; cold-start-friendly DMA order:
            # sync: dt, first weights; scalar: x(c0), y(c0), then c1
            dt_sb = rpool.tile([1, panel], F32, name="dt_sb", tag="dtr", bufs=1)
            nc.sync.dma_start(dt_sb[:], dtr[0:1, b0 : b0 + panel])

            # per-batch dt gates first: tiny ACT ops must precede the input
            # DMA flood in the ACT FIFO, else bc gates arrive ~40us late
            sg1 = rpool.tile([1, panel], F32, name="sg1", tag="sg")
            nc.scalar.activation(
                sg1[:], dt_sb[:], AF.Sigmoid,
                bias=bias_sb[0:1, 4 * NJT : 4 * NJT + 1], scale=wdt00,
            )
            sg2 = rpool.tile([1, panel], F32, name="sg2", tag="sg")
            nc.scalar.activation(
                sg2[:], dt_sb[:], AF.Sigmoid,
                bias=bias_sb[0:1, 4 * NJT + 1 : 4 * NJT + 2], scale=wdt10,
            )
            bc1 = bcpool.tile([P, panel], F32, name="bc1", tag="bc1")
            nc.gpsimd.partition_broadcast(bc1[:], sg1[0:1, :])
            bc2 = bcpool.tile([P, panel], F32, name="bc2", tag="bc2")
            nc.gpsimd.partition_broadcast(bc2[:], sg2[0:1, :])

            x_t = [[None] * nch for _ in range(NKT)]
            y_t = [[None] * nch for _ in range(NHT)]

            def load_x(kt, c):
                xt_ = xpool.tile([P, chunk], mm_dt, name="xt", tag="xt")
                nc.scalar.dma_start(xt_[:], xT[kt * P : (kt + 1) * P, col(c)])
                x_t[kt][c] = xt_

            def load_y(kt, c):
                yt_ = ypool.tile([P, chunk], mm_dt, name="yt", tag="yt")
                nc.sync.dma_start(yt_[:], yT[kt * P : (kt + 1) * P, col(c)])
                y_t[kt][c] = yt_

            def load_w(Wsrc, jt, name):
                w_sb = wpool.tile([P, K + H], mm_dt, name=name, tag="w")
                nc.sync.dma_start(w_sb[:, 0:K], Wsrc[jt][:, 0:K])
                nc.scalar.dma_start(w_sb[:, K : K + H], Wsrc[jt][:, K : K + H])
                return w_sb

            def load_w_half(Wsrc, jt, w_sb, half):
                eng = nc.sync if half == 0 else nc.scalar
                lo = 0 if half == 0 else K
                hi = K if half == 0 else K + H
                eng.dma_start(w_sb[:, lo:hi], Wsrc[jt][:, lo:hi])

            # staged cold-start: the ih halves (sync) land before y(c0),
            # the hh halves (scalar) after x(c0); two jt's worth prestaged
            n_pre = min(2, NJT)
            pre_w = []
            for jt in range(n_pre):
                wd2_sb = wpool.tile([P, K + H], mm_dt, name="wd2_sb", tag="w")
                wy_sb = wpool.tile([P, K + H], mm_dt, name="wy_sb", tag="w")
                pre_w.append((wd2_sb, wy_sb))
            for idx in range(max(n_pre, nch)):
                if idx < n_pre:
                    load_w_half(Wd2, idx, pre_w[idx][0], 0)
                    load_w_half(Wy, idx, pre_w[idx][1], 0)
                if idx < nch:
                    for kt in range(NKT):
                        load_x(kt, idx)
                if idx < n_pre:
                    load_w_half(Wd2, idx, pre_w[idx][0], 1)
                    load_w_half(Wy, idx, pre_w[idx][1], 1)
                if idx < nch:
                    for kt in range(NHT):
                        load_y(kt, idx)

            def accum_group(ps, w_sb, rhs_a, rhs_b):
                """16-matmul accumulation: sum_k Wa[k].T@a[k] + Wb[k].T@b[k]."""
                n_a = len(rhs_a)
                for kt in range(n_a):
                    nc.tensor.matmul(
                        ps[:],
                        lhsT=w_sb[:, kt * P : (kt + 1) * P],
                        rhs=rhs_a[kt][:],
                        start=(kt == 0),
                        stop=False,
                    )
                n_b = len(rhs_b)
                for kt in range(n_b):
                    nc.tensor.matmul(
                        ps[:],
                        lhsT=w_sb[:, K + kt * P : K + (kt + 1) * P],
                        rhs=rhs_b[kt][:],
                        start=False,
                        stop=(kt == n_b - 1),
                    )

            # ---- phase B: d2 + y gates -> z_new ----
            zn_t = []
            for jt in range(NJT):
                if jt < n_pre:
                    wd2_sb, wy_sb = pre_w[jt]
                else:
                    wd2_sb = load_w(Wd2, jt, "wd2_sb")
                    wy_sb = load_w(Wy, jt, "wy_sb")
                znr = znpool.tile([P, panel], mm_dt, name="znr", tag="zn")
                zn_t.append(znr)
                for c in range(nch):
                    cs = slice(c * chunk, (c + 1) * chunk)
                    z_sb = zpool.tile([P, chunk], F32, name="z_sb", tag="z")
                    nc.gpsimd.dma_start(z_sb[:], zT[jt * P : (jt + 1) * P, col(c)])

                    ps1 = pspool.tile([P, chunk], F32, name="ps1", tag="ps")
                    accum_group(ps1, wd2_sb, [x_t[k][c] for k in range(NKT)],
                                [y_t[k][c] for k in range(NHT)])
                    s2 = apool.tile([P, chunk], F32, name="s2", tag="act")
                    nc.scalar.activation(s2[:], ps1[:], AF.Sigmoid, bias=bias_ap(0, jt), scale=1.0)

                    ps2 = pspool.tile([P, chunk], F32, name="ps2", tag="ps")
                    accum_group(ps2, wy_sb, [x_t[k][c] for k in range(NKT)],
                                [y_t[k][c] for k in range(NHT)])
                    tz = apool.tile([P, chunk], F32, name="tz", tag="act")
                    nc.scalar.activation(tz[:], ps2[:], AF.Tanh, bias=bias_ap(1, jt), scale=1.0)

                    ms2 = dpool.tile([P, chunk], F32, name="ms2", tag="dve")
                    nc.vector.tensor_mul(ms2[:], s2[:], bc2[:, cs])
                    dlt = dpool.tile([P, chunk], F32, name="dlt", tag="dve")
                    nc.vector.tensor_sub(dlt[:], tz[:], z_sb[:])
                    prd = dpool.tile([P, chunk], F32, name="prd", tag="dve")
                    nc.vector.tensor_mul(prd[:], ms2[:], dlt[:])
                    znc = opool.tile([P, chunk], F32, name="znc", tag="znc")
                    nc.vector.tensor_add(znc[:], prd[:], z_sb[:])
                    nc.sync.dma_start(
                        z_newT[jt * P : (jt + 1) * P, col(c)], znc[:]
                    )
                    # rounding cast into the resident fp32r tile for GEMM3
                    nc.gpsimd.dma_start(znr[:, cs], znc[:])

            # ---- phase C: d1 gate + (i_z + z_new @ W_z.T) -> y_new ----
            for jt in range(NJT):
                wd1_sb = load_w(Wd1, jt, "wd1_sb")
                wg3_sb = load_w(Wg3, jt, "wg3_sb")
                for c in range(nch):
                    cs = slice(c * chunk, (c + 1) * chunk)
                    ps3 = pspool.tile([P, chunk], F32, name="ps3", tag="ps")
                    accum_group(ps3, wd1_sb, [x_t[k][c] for k in range(NKT)],
                                [y_t[k][c] for k in range(NHT)])
                    s1 = apool.tile([P, chunk], F32, name="s1", tag="act")
                    nc.scalar.activation(s1[:], ps3[:], AF.Sigmoid, bias=bias_ap(2, jt), scale=1.0)

                    ps4 = pspool.tile([P, chunk], F32, name="ps4", tag="ps")
                    accum_group(ps4, wg3_sb, [x_t[k][c] for k in range(NKT)],
                                [zn_t[h][:, cs] for h in range(NHT)])
                    u = apool.tile([P, chunk], F32, name="u", tag="act")
                    nc.scalar.activation(u[:], ps4[:], AF.Tanh, bias=bias_ap(3, jt), scale=1.0)

                    # yn = (y - ms1*y) + ms1*u; the first two ops only
                    # need s1, so just two DVE ops trail the final tanh
                    ms1 = dpool.tile([P, chunk], F32, name="ms1", tag="dve")
                    nc.vector.tensor_mul(ms1[:], s1[:], bc1[:, cs])
                    my = dpool.tile([P, chunk], F32, name="my", tag="dve")
                    nc.vector.tensor_mul(my[:], ms1[:], f32v(y_t[jt][c][:]))
                    wyp = dpool.tile([P, chunk], F32, name="wyp", tag="dve")
                    nc.vector.tensor_sub(wyp[:], f32v(y_t[jt][c][:]), my[:])
                    mu = dpool.tile([P, chunk], F32, name="mu", tag="dve")
                    nc.vector.tensor_mul(mu[:], ms1[:], u[:])
                    yn = opool.tile([P, chunk], F32, name="yn", tag="yn")
                    nc.vector.tensor_add(yn[:], wyp[:], mu[:])
                    nc.scalar.dma_start(
                        y_newT[jt * P : (jt + 1) * P, col(c)], yn[:]
                    )

    nc.compile()
    return nc


def _pack_pair(Wa, Wb):
    """[jt, kin, kt*P+j] stationary-block packing of two row-major [out, in]
    weight matrices (lhsT blocks: lhsT[kin, j] = W[jt*P+j, kt*P+kin])."""
    def pack(W):
        O, I = W.shape
        njt, nkt = O // P, I // P
        return (
            W.reshape(njt, P, nkt, P).transpose(0, 3, 2, 1).reshape(njt, P, I)
        )
    A = pack(Wa)
    B = pack(Wb)
    return np.ascontiguousarray(np.concatenate([A, B], axis=2), dtype=np.float32)


def pack_host_inputs(x, y, z, dt, W_ih, b_ih, W_hh, b_hh, W_z, b_z, b_dt, n_cores):
    """Shard batch across cores; pre-transpose activations; pack weights."""
    B, K = x.shape
    H = y.shape[1]
    NJT = H // P
    Bs = B // n_cores

    xT = np.ascontiguousarray(x.T)
    yT = np.ascontiguousarray(y.T)
    zT = np.ascontiguousarray(z.T)
    dtrow = np.ascontiguousarray(dt.reshape(1, B))

    Wd2 = _pack_pair(W_ih[H : 2 * H], W_hh[H : 2 * H])
    Wy = _pack_pair(W_ih[3 * H : 4 * H], W_hh[2 * H : 3 * H])
    Wd1 = _pack_pair(W_ih[0:H], W_hh[0:H])
    Wg3 = _pack_pair(W_ih[2 * H : 3 * H], W_z)

    def bias_cols(bvec):
        return bvec.reshape(NJT, P).T  # [P, NJT]

    bdt_cols = np.zeros((P, 2), np.float32)
    bdt_cols[0, 0] = b_dt[0]
    bdt_cols[0, 1] = b_dt[1]
    biasP = np.ascontiguousarray(
        np.concatenate(
            [
                bias_cols(b_ih[H : 2 * H] + b_hh[H : 2 * H]),
                bias_cols(b_ih[3 * H : 4 * H] + b_hh[2 * H : 3 * H]),
                bias_cols(b_ih[0:H] + b_hh[0:H]),
                bias_cols(b_ih[2 * H : 3 * H] + b_z),
                bdt_cols,
            ],
            axis=1,
        ),
        dtype=np.float32,
    )

    in_maps = []
    for c in range(n_cores):
        cs = slice(c * Bs, (c + 1) * Bs)
        in_maps.append(
            {
                "xT": np.ascontiguousarray(xT[:, cs]),
                "yT": np.ascontiguousarray(yT[:, cs]),
                "zT": np.ascontiguousarray(zT[:, cs]),
                "dtr": np.ascontiguousarray(dtrow[:, cs]),
                "Wd2": Wd2,
                "Wy": Wy,
                "Wd1": Wd1,
                "Wg3": Wg3,
                "biasP": biasP,
            }
        )
    return in_maps


def kernel(x, y, z, dt, W_ih, b_ih, W_hh, b_hh, W_z, b_z, W_dt, b_dt):
    x = np.asarray(x, np.float32)
    y = np.asarray(y, np.float32)
    z = np.asarray(z, np.float32)
    dt = np.asarray(dt, np.float32)
    W_ih = np.asarray(W_ih, np.float32)
    b_ih = np.asarray(b_ih, np.float32)
    W_hh = np.asarray(W_hh, np.float32)
    b_hh = np.asarray(b_hh, np.float32)
    W_z = np.asarray(W_z, np.float32)
    b_z = np.asarray(b_z, np.float32)
    W_dt = np.asarray(W_dt, np.float32)
    b_dt = np.asarray(b_dt, np.float32)

    B, K = x.shape
    H = y.shape[1]
    Bs = B // N_CORES

    in_maps = pack_host_inputs(
        x, y, z, dt, W_ih, b_ih, W_hh, b_hh, W_z, b_z, b_dt, N_CORES
    )
    nc = build_nc(
        K,
        H,
        Bs,
        panel=1024,
        chunk=512,
        wdt00=float(W_dt[0, 0]),
        wdt10=float(W_dt[1, 0]),
    )
    import os

    trace = os.environ.get("LEM_TRACE", "0") == "1"
    tmpdir = os.environ.get("LEM_TMPDIR") or None
    res = run_bass_kernel_spmd(
        nc, in_maps, list(range(N_CORES)), trace=trace, tmpdir=tmpdir
    )
    global LAST_RESULTS
    LAST_RESULTS = res
    y_newT = np.concatenate([r["y_newT"] for r in res.results], axis=1)
    z_newT = np.concatenate([r["z_newT"] for r in res.results], axis=1)
    return (
        np.ascontiguousarray(y_newT.T, dtype=np.float32),
        np.ascontiguousarray(z_newT.T, dtype=np.float32),
    )
